# revision 15
# baseline (speedup 1.0000x reference)
"""ConvGraphLayer kernel for 8 Trainium2 NeuronCores.

Computes: relu(concat([x, (adj @ x) / (nn + eps)], -1) @ fc_w.T + fc_b)

Strategy (1-D node/data parallel, per the row-shard hint):
  - Row-shard adj and num_neighbors across 8 cores (1250 rows each). The
    adjacency stream dominates, so the kernel is HBM-DMA-bound; everything
    else is arranged to keep that stream saturated and the post-stream tail
    short.
  - adj is staged host-side as centered float8_e3m4: adj ~ Q(adj - 0.5) + 0.5.
    The rank-1 remainder 0.5*colsum(x) is a host-computed 64-float constant
    applied inside the epilogue normalization; 1/(nn+eps) is host precomputed
    (a [1,1250] constant) and partition-broadcast on device. x is also e3m4
    (measured end-to-end rel err 1.02e-2 vs the 2e-2 gate; adj+x fp8
    quantization dominates the error).
  - The PE array column dim is only half used by the 64-wide x stationary, so
    k-tiles are processed in PAIRS on two concurrent 128x64 column tiles:
    even k-tiles on tile_position (0,0) -> PSUM[0:64], odd on (0,64) ->
    PSUM[64:128]. This doubles matmul throughput (2 moving cols/cycle
    aggregate), giving the PE ~1.7x headroom over the DMA stream even at the
    cold (1.2GHz) clock, so HAM throttling can never make the PE the
    critical path.
  - k-tiles are 128 rows (10000 zero-padded to 79*128): a 125-row layout
    measured collapsing the DMA descriptor fan-out to 5 of 16 SDMA engines
    (145GB/s instead of ~360GB/s).
  - Epilogue: ONE 128-lane DVE op per chunk computes cat2 = (nb + h)*recip
    over both halves (h = [0.5*colsum; 0]); the halves are then SUMMED BY THE
    FC CONTRACTION with stationary [W_nb^T; W_nb^T]. The x-self FC pass
    (stationary [W_nb^T; W_x^T] against [zeros; x_self]) accumulates into the
    same PSUM early, during the adj stream, so the post-stream tail is only
    (2 k-tiles of matmul) -> STT -> FC -> ReLU -> store per chunk.
  - DMA: sync ring = x head chunk + the pure adj stream as a few ~1.5MB
    fully-contiguous transfers (measured ~360GB/s, at the per-core HBM cap).
    Scalar ring (the second HWDGE ring) = all small tensors, the x tail, the
    16-real-row k-tile 78 (a partial-partition transfer that stalls a ring
    ~0.5us), and one output store - so none of them ever pause the adj
    stream. The final k-tiles 74-77 are the last bytes on the wire and their
    matmuls issue chunk-major so each PSUM chunk's epilogue overlaps the
    rest.
  - cat2/x_self/fc_w/output are bf16; output is upcast to fp32 on the host.
"""

import sys

import numpy as np

try:
    import concourse.bacc as bacc
except ImportError:  # concourse ships in the container image, not on PyPI
    for _p in ("/opt/trn_rl_repo", "/root/.axon_site/_ro/trn_rl_repo"):
        if _p not in sys.path:
            sys.path.append(_p)
    import concourse.bacc as bacc

import ml_dtypes
import concourse.mybir as mybir
import concourse.tile as tile
from concourse import bass_utils

N_NODES = 10000
F = 64
H = 64
EPS = 1e-7
N_CORES = 8
ROWS = N_NODES // N_CORES  # 1250 rows per core

F32 = mybir.dt.float32
BF16 = mybir.dt.bfloat16
F8E3 = mybir.dt.float8e3

KT = 79                    # k-tiles (contraction), zero-padded 10000 -> 10112
KROWS = 128
NPAD = KT * KROWS          # 10112
XGROUPS = 80
XFREE = XGROUPS * F        # 5120
XHEAD = 1024               # first x DMA covers k-groups 0..15 (+ warmup reads)
# i-chunks; smallest last so the final ACT+store tail is short.
ICHUNKS = [(0, 512), (512, 482), (994, 256)]
# adjacency DMA slices (start k-tile, count), in queue order: big steady-state
# transfers for DMA efficiency, graduated tail so the epilogue starts early.
# The 16-real-row tile 78 is queued BEFORE the final slices so the last bytes
# on the wire are full tiles and the post-stream chain is short.
SLICES = [(0, 10), (10, 10), (20, 10), (30, 10), (40, 10), (50, 10),
          (60, 8), (68, 6), (78, 1), (74, 2), (76, 2)]
assert sorted(kt for st, cnt in SLICES for kt in range(st, st + cnt)) == list(range(KT))
TAIL_PAIR_MAJOR = 74       # k-tiles >= this are issued chunk-major
PADR = N_NODES - (KT - 1) * KROWS  # real rows in the last k-tile (16)

TRACE = False
TRACE_KWARGS = {}
LAST_RESULTS = None

_PROGRAM = None


def _build_body(tc, nc, adjq, x_tiled, x_selfT, recip_row, halfs_d, fc_wT,
                fc_w1bT, fc_b_col, out_rowsT):
    RELU = mybir.ActivationFunctionType.Relu
    ADD = mybir.AluOpType.add
    MULT = mybir.AluOpType.mult

    # kt -> (slice idx, local kt) map
    kt_map = {}
    for si, (st, cnt) in enumerate(SLICES):
        for lk in range(cnt):
            kt_map[st + lk] = (si, lk)

    with (
        tc.tile_pool(name="const", bufs=1) as cpool,
        tc.tile_pool(name="psum", bufs=1, space="PSUM") as ppool,
    ):
        x_sb = cpool.tile([128, XFREE], F8E3, name="x_sb", tag="x_sb")
        adj_sb = [
            cpool.tile([128, cnt * ROWS], F8E3, name=f"adj_sb{si}", tag=f"adj_sb{si}")
            for si, (st, cnt) in enumerate(SLICES)
        ]
        cat2 = cpool.tile([128, ROWS], BF16, name="cat2", tag="cat2")
        xz_sb = cpool.tile([128, ROWS], BF16, name="xz_sb", tag="xz_sb")
        recip_sb1 = cpool.tile([128, ROWS], F32, name="recip_sb1", tag="recip_sb1")
        recip_sb = cpool.tile([128, ROWS], F32, name="recip_sb", tag="recip_sb")
        fcw_sb = cpool.tile([2 * F, H], BF16, name="fcw_sb", tag="fcw_sb")
        fcw1b_sb = cpool.tile([2 * F, H], BF16, name="fcw1b_sb", tag="fcw1b_sb")
        fcb_sb = cpool.tile([H, 1], F32, name="fcb_sb", tag="fcb_sb")
        halfs_sb = cpool.tile([128, 1], F32, name="halfs_sb", tag="halfs_sb")
        outT_sb = cpool.tile([H, ROWS], BF16, name="outT_sb", tag="outT_sb")

        nb_ps = [
            ppool.tile([128, w], F32, name=f"nb_ps{ci}", tag=f"nb_ps{ci}")
            for ci, (_, w) in enumerate(ICHUNKS)
        ]
        oT_ps = [
            ppool.tile([128, w], F32, name=f"oT_ps{ci}", tag=f"oT_ps{ci}")
            for ci, (_, w) in enumerate(ICHUNKS)
        ]

        # ---- PE warmup: ramp the tensor-engine p-state during the DMA head
        # (fp32 4-pass matmuls = long busy time per instruction)
        scratch = cpool.tile([128, 576], F32, name="scratch", tag="scratch")
        nc.vector.memset(scratch[:, :], 0.0)
        for tp in ((0, 0), (0, 64)):
            nc.tensor.matmul(
                nb_ps[0][tp[1] : tp[1] + 64, :], scratch[:, 0:64],
                scratch[:, 64:576], start=True, stop=True, tile_position=tp,
            )
        # x-self FC pass contracts [W_nb^T; W_x^T] against [zeros; x_self].
        nc.vector.memset(xz_sb[0:64, :], 0.0)
        # (engine partition access must be 32-aligned: zero the whole tile,
        # the 16-real-row DMA then overwrites partitions 0:16)
        pad_si = next(si for si, (st, cnt) in enumerate(SLICES) if st == KT - 1)
        nc.vector.memset(adj_sb[pad_si][:, :], 0.0)

        # ---- sync-ring DMA queue: x head chunk, then the pure adj stream ----
        nc.sync.dma_start(x_sb[:, 0:XHEAD], x_tiled[:, 0:XHEAD])
        for si, (st, cnt) in enumerate(SLICES):
            if si == pad_si:
                continue  # 16-row tile 78 rides the scalar ring (below): its
                # partial-partition transfer stalls the HWDGE ring ~0.5us.
            nc.sync.dma_start(
                adj_sb[si][:, :], adjq[:, st * ROWS : (st + cnt) * ROWS]
            )
        # ---- scalar-ring DMA queue: everything small + x tail ----
        nc.scalar.dma_start(xz_sb[F : 2 * F, :], x_selfT[:, :])
        st78 = SLICES[pad_si][0]
        nc.scalar.dma_start(adj_sb[pad_si][0:PADR, :], adjq[0:PADR, st78 * ROWS :])
        nc.scalar.dma_start(x_sb[:, XHEAD:], x_tiled[:, XHEAD:])
        nc.scalar.dma_start(recip_sb1[0:1, :], recip_row[:, :])
        nc.scalar.dma_start(halfs_sb[:, :], halfs_d[:, :])
        nc.scalar.dma_start(fcb_sb[:, :], fc_b_col[:, :])
        nc.scalar.dma_start(fcw_sb[:, :], fc_wT[:, :])
        nc.scalar.dma_start(fcw1b_sb[:, :], fc_w1bT[:, :])

        # ---- small precompute ----
        nc.gpsimd.partition_broadcast(recip_sb[:, :], recip_sb1[0:1, :])
        # bf16 re-warmups gated on the x head chunk: the PE idles > the HAM
        # MID window while x+slice0 stream in, so re-warm just before k-tile 0.
        for tp in ((0, 0), (0, 64)):
            nc.tensor.matmul(
                nb_ps[0][tp[1] : tp[1] + 64, 0:512], x_sb[:, 0:64],
                x_sb[:, 64:576], start=True, stop=True, tile_position=tp,
            )

        # ---- main stream: k-tile pairs on two concurrent 128x64 col tiles ----
        # even k-tiles -> tile (0,0) / PSUM[0:64]; odd -> (0,64) / PSUM[64:128].
        # KT=79 is odd: tile 78 runs solo on tile (0,0).
        def kt_mm(kt, ci):
            o, w = ICHUNKS[ci]
            half = kt % 2
            si, lk = kt_map[kt]
            # program-order-last accumulating matmul per column tile: the
            # 16-row tile 78 is issued mid-stream (its data lands early), so
            # T0 ends at 76 and T1 at 77.
            last = KT - 3 if half == 0 else KT - 2
            nc.tensor.matmul(
                nb_ps[ci][64 * half : 64 * half + 64, :],
                x_sb[:, kt * F : (kt + 1) * F],
                adj_sb[si][:, lk * ROWS + o : lk * ROWS + o + w],
                start=(kt == half),
                stop=(kt == last),
                tile_position=(0, 64 * half),
            )

        first_pairs = SLICES[0][1] // 2
        for p in range(first_pairs):
            for ci in range(len(ICHUNKS)):
                kt_mm(2 * p, ci)
                kt_mm(2 * p + 1, ci)
        # x-self FC pass, early: runs on the PE during the adj stream (its
        # inputs land on the scalar ring ~10us in; slice-0 pairs above keep
        # the PE FIFO from stalling on it).
        for ci, (o, w) in enumerate(ICHUNKS):
            nc.tensor.matmul(
                oT_ps[ci][0:64, :], fcw_sb[:, :], xz_sb[:, o : o + w],
                start=True, stop=False, tile_position=(0, 0),
            )
        for p in range(first_pairs, 37):
            for ci in range(len(ICHUNKS)):
                kt_mm(2 * p, ci)
                kt_mm(2 * p + 1, ci)
        # tile 78 (16 real rows, 20KB slice, lands mid-stream): run it here so
        # the post-stream chain is only tiles 76/77 per chunk.
        for ci in range(len(ICHUNKS)):
            kt_mm(KT - 1, ci)
        for ci in range(len(ICHUNKS)):
            kt_mm(74, ci)
            kt_mm(75, ci)
        for ci in range(len(ICHUNKS)):
            kt_mm(76, ci)
            kt_mm(77, ci)

        # ---- epilogue, chunk-pipelined ----
        for ci, (o, w) in enumerate(ICHUNKS):
            # cat2 = (nb + [0.5*colsum; 0]) * recip over both halves at once
            nc.vector.scalar_tensor_tensor(
                cat2[:, o : o + w],
                nb_ps[ci][:, :],
                halfs_sb[:, 0:1],
                recip_sb[:, o : o + w],
                op0=ADD,
                op1=MULT,
            )
            # halves summed by the contraction: stationary [W_nb^T; W_nb^T]
            nc.tensor.matmul(
                oT_ps[ci][0:64, :], fcw1b_sb[:, :], cat2[:, o : o + w],
                start=False, stop=True, tile_position=(0, 0),
            )
            nc.scalar.activation(
                outT_sb[:, o : o + w], oT_ps[ci][0:64, :], RELU, bias=fcb_sb[:, :]
            )
            # alternate rings so consecutive stores do not FIFO-serialize
            eng = nc.scalar if ci == 1 else nc.sync
            eng.dma_start(out_rowsT[:, o : o + w], outT_sb[:, o : o + w])


def _get_program():
    global _PROGRAM
    if _PROGRAM is not None:
        return _PROGRAM
    nc = bacc.Bacc("TRN2", target_bir_lowering=False, debug=False)
    adjq = nc.dram_tensor("adjq", [KROWS, KT * ROWS], F8E3, kind="ExternalInput").ap()
    x_tiled = nc.dram_tensor("x_tiled", [KROWS, XFREE], F8E3, kind="ExternalInput").ap()
    x_selfT = nc.dram_tensor("x_selfT", [F, ROWS], BF16, kind="ExternalInput").ap()
    recip_row = nc.dram_tensor("recip_row", [1, ROWS], F32, kind="ExternalInput").ap()
    halfs_d = nc.dram_tensor("halfs_d", [128, 1], F32, kind="ExternalInput").ap()
    fc_wT = nc.dram_tensor("fc_wT", [2 * F, H], BF16, kind="ExternalInput").ap()
    fc_w1bT = nc.dram_tensor("fc_w1bT", [2 * F, H], BF16, kind="ExternalInput").ap()
    fc_b_col = nc.dram_tensor("fc_b_col", [H, 1], F32, kind="ExternalInput").ap()
    out_rowsT = nc.dram_tensor("out_rowsT", [H, ROWS], BF16, kind="ExternalOutput").ap()

    with tile.TileContext(nc) as tc:
        _build_body(tc, nc, adjq, x_tiled, x_selfT, recip_row, halfs_d, fc_wT,
                    fc_w1bT, fc_b_col, out_rowsT)
    nc.compile()
    _PROGRAM = nc
    return nc


def kernel(x, adj_matrix, num_neighbors, fc_w, fc_b):
    global LAST_RESULTS
    x = np.ascontiguousarray(np.asarray(x, dtype=np.float32))
    adj_matrix = np.asarray(adj_matrix, dtype=np.float32)
    num_neighbors = np.asarray(num_neighbors, dtype=np.float32)
    fc_w = np.asarray(fc_w, dtype=np.float32)
    fc_b = np.asarray(fc_b, dtype=np.float32)
    assert adj_matrix.shape == (N_NODES, N_NODES)

    # Host staging (layout + dtype prep): centered e3m4 quantization of adj,
    # transposed so the contraction dim lands on SBUF partitions, pre-tiled so
    # each core's shard is one contiguous [128, 79*1250] block.
    adjq8 = (adj_matrix - np.float32(0.5)).astype(ml_dtypes.float8_e3m4)
    Mq = np.zeros((NPAD, N_NODES), dtype=ml_dtypes.float8_e3m4)
    Mq[:N_NODES, :] = adjq8.T
    Tq = Mq.reshape(KT, KROWS, N_NODES)

    xb = x.astype(ml_dtypes.float8_e3m4)
    xp = np.zeros((NPAD, F), dtype=ml_dtypes.float8_e3m4)
    xp[:N_NODES] = xb
    x_tiled = np.zeros((KROWS, XFREE), dtype=ml_dtypes.float8_e3m4)
    x_tiled[:, : KT * F] = (
        xp.reshape(KT, KROWS, F).transpose(1, 0, 2).reshape(KROWS, KT * F)
    )

    xT16 = np.ascontiguousarray(x.T.astype(ml_dtypes.bfloat16))  # [F, N]
    halfs128 = np.zeros((128, 1), dtype=np.float32)
    halfs128[:F, 0] = (0.5 * x.astype(np.float64).sum(axis=0)).astype(np.float32)
    recip_full = (1.0 / (num_neighbors + np.float32(EPS))).astype(np.float32)
    fc_wT_full = np.ascontiguousarray(
        np.concatenate([fc_w[:, F:], fc_w[:, :F]], axis=1).T.astype(ml_dtypes.bfloat16)
    )
    fc_w1bT_full = np.ascontiguousarray(
        np.concatenate([fc_w[:, F:], fc_w[:, F:]], axis=1).T.astype(ml_dtypes.bfloat16)
    )
    fcb_col = np.ascontiguousarray(fc_b).reshape(H, 1)

    in_maps = []
    for c in range(N_CORES):
        sl = slice(c * ROWS, (c + 1) * ROWS)
        A = np.ascontiguousarray(
            Tq[:, :, sl].transpose(1, 0, 2).reshape(KROWS, KT * ROWS)
        )
        in_maps.append(
            {
                "adjq": A,
                "x_tiled": x_tiled,
                "x_selfT": np.ascontiguousarray(xT16[:, sl]),
                "recip_row": np.ascontiguousarray(recip_full[sl]).reshape(1, ROWS),
                "halfs_d": halfs128,
                "fc_wT": fc_wT_full,
                "fc_w1bT": fc_w1bT_full,
                "fc_b_col": fcb_col,
            }
        )

    nc = _get_program()
    results = bass_utils.run_bass_kernel_spmd(
        nc,
        in_maps,
        core_ids=list(range(N_CORES)),
        trace=TRACE,
        **TRACE_KWARGS,
    )
    LAST_RESULTS = results
    outs = [
        results.results[c]["out_rowsT"].T.astype(np.float32) for c in range(N_CORES)
    ]
    return np.ascontiguousarray(np.concatenate(outs, axis=0))
